# revision 31
# baseline (speedup 1.0000x reference)
"""Trainium2 Bass kernel for nn_MultiHeadAttn (B=4, S=2048, D=1024, H=16).

Sharding: 8 cores = 4 batches x 2 head-groups (tensor-parallel over heads).
Each core computes one batch's attention for 8 of 16 heads (512 of 1024
feature dims) and a partial output projection; the host sums the two
head-group partials per batch (the "all-reduce" of row-parallel Wo).

v2 dataflow (fp16 matmuls, fp32 PSUM):
  - qT/kT are DMA'd ONCE into resident SBUF chunks ([128, 2048] each),
    issued t-slice-major so the first projections overlap the tail of the
    load; vT streams through [128, 512] window tiles.  This removes the 4x
    re-load of q/k windows the m-granular projection fillers used to pay.
  - The exp chain (ScalarE, ~1.06us per [128,1024] tile, 256 tiles) and
    the PE matmul stream (~273us warm) are both ~270us/core: the kernel
    interleaves them so each shadows the other.  Scores for iteration k+1
    and the lag-4 attn@V pair are emitted under exp(k)'s shadow; deferred
    projection m-chunks (1.7us each) fill the remaining PE slack 3 per
    tile; the output projection is emitted as per-m chunks (0.87us), 8
    per tile, inside the j=3 row so it never bunches at tile boundaries.
  - ys ([65,512] fp32 PSUM: 64 attn rows + ones-row sums) is evacuated to
    SBUF by one DVE copy right after the accumulation stops, freeing the
    PSUM bank for the next tile; the softmax division (gpsimd broadcast +
    DVE reciprocal + multiply) then runs entirely from SBUF off the
    critical path.
  - PSUM: 2x proj/V/out accum banks + 2x scores (2 banks each) + 2x ys
    = 8 banks exactly.
"""
import numpy as np

B, S, D = 4, 2048, 1024
H = 16
DK = 64
G = 2              # head groups (tensor-parallel factor)
DL = D // G        # 512 local feature dims per core
NHL = H // G       # 8 local heads
NJ = NHL // 2      # 4 head pairs
NT = S // 512      # 4 token tiles of 512
NKC = S // 128     # 16 k-token chunks of 128
NDC = D // 128     # 8 d_in chunks
NM = DL // 128     # 4 local out chunks
NMO = D // 128     # 8 output d chunks

_CACHED = {}


def _build_nc():
    import concourse.bass as bass
    import concourse.tile as tile
    from concourse import bacc, mybir

    FP32 = mybir.dt.float32
    FP16 = mybir.dt.float16
    AF = mybir.ActivationFunctionType
    ts = bass.ts

    nc = bacc.Bacc(None, target_bir_lowering=False, debug=False)

    # inputs are repacked host-side into the exact SBUF layouts so every
    # DMA is a plain [128, n] copy with 8-32KB contiguous descriptors
    # (1KB descriptors cap a HWDGE ring at ~70-100 GB/s of generation)
    qT_d = nc.dram_tensor("qT", [128, NT * NDC * 512], FP16,
                          kind="ExternalInput")
    kT_d = nc.dram_tensor("kT", [128, NT * NDC * 512], FP16,
                          kind="ExternalInput")
    vT_d = nc.dram_tensor("vT", [128, NT * NDC * 512], FP16,
                          kind="ExternalInput")
    wqT_d = nc.dram_tensor("wqT", [128, NDC * DL], FP16,
                           kind="ExternalInput")
    wkT_d = nc.dram_tensor("wkT", [128, NDC * DL], FP16,
                           kind="ExternalInput")
    wvT_d = nc.dram_tensor("wvT", [128, NDC * DL], FP16,
                           kind="ExternalInput")
    woT_d = nc.dram_tensor("woT", [128, NJ * D], FP16,
                           kind="ExternalInput")
    bq_d = nc.dram_tensor("bq", [128, NM], FP32, kind="ExternalInput")
    bk_d = nc.dram_tensor("bk", [128, NM], FP32, kind="ExternalInput")
    bo_d = nc.dram_tensor("bo", [128, NMO], FP32, kind="ExternalInput")
    out_d = nc.dram_tensor("outT", [D, S], FP32, kind="ExternalOutput")

    with tile.TileContext(nc) as tc:
        with (
            tc.tile_pool(name="const", bufs=1) as const,
            tc.tile_pool(name="wflat", bufs=1) as wflat,
            tc.tile_pool(name="wop", bufs=1) as wop,
            tc.tile_pool(name="xfull", bufs=1) as xfull,
            tc.tile_pool(name="vwinp", bufs=8) as vwinp,
            tc.tile_pool(name="kres", bufs=1) as kres,
            tc.tile_pool(name="qtp", bufs=12) as qtp,
            tc.tile_pool(name="xp", bufs=16) as xp,
            tc.tile_pool(name="vaug", bufs=1) as vaug,
            tc.tile_pool(name="ppool", bufs=5) as ppool,
            tc.tile_pool(name="ybufp", bufs=4) as ybufp,
            tc.tile_pool(name="small", bufs=2) as small,
            tc.tile_pool(name="outstp", bufs=3) as outstp,
            tc.tile_pool(name="ps_mm", bufs=2, space="PSUM") as ps_mm,
            tc.tile_pool(name="ps_s", bufs=2, space="PSUM") as ps_s,
            tc.tile_pool(name="ps_y", bufs=2, space="PSUM") as ps_y,
        ):
            # ---- constants
            onescols = const.tile([128, NHL, 1], FP16, name="onescols")
            nc.vector.memset(onescols[:], 1.0)

            # ---- weight tiles: one flat tile per weight matrix, loaded by
            # a single strided DMA (few big ring entries instead of 8 small
            # ones each — the head phase is gated by DMA ring serialization)
            wk_big = wflat.tile([128, NDC * DL], FP16, name="wk_big")
            wq_big = wflat.tile([128, NDC * DL], FP16, name="wq_big")
            wv_big = wflat.tile([128, NDC * DL], FP16, name="wv_big")
            wo_big = wop.tile([128, NJ * D], FP16, name="wo_big")
            wkv = wk_big[:].rearrange("p (c n) -> p c n", n=DL)
            wqv = wq_big[:].rearrange("p (c n) -> p c n", n=DL)
            wvv = wv_big[:].rearrange("p (c n) -> p c n", n=DL)
            wov = wo_big[:].rearrange("p (c n) -> p c n", n=D)
            wk_sb = [wkv[:, kc, :] for kc in range(NDC)]
            wq_sb = [wqv[:, kc, :] for kc in range(NDC)]
            wv_sb = [wvv[:, kc, :] for kc in range(NDC)]
            wo_sb = [wov[:, jc, :] for jc in range(NJ)]

            # ---- resident activation sources / destinations
            kfull_big = xfull.tile([128, NDC * S], FP16, name="kfull")
            qfull_big = xfull.tile([128, NDC * S], FP16, name="qfull")
            k3 = kfull_big[:].rearrange("p (t c s) -> p t c s", t=NT, s=512)
            q3 = qfull_big[:].rearrange("p (t c s) -> p t c s", t=NT, s=512)
            kT_v = kT_d[:].rearrange("p (t c s) -> p t c s", t=NT, s=512)
            qT_v = qT_d[:].rearrange("p (t c s) -> p t c s", t=NT, s=512)
            vT_v = vT_d[:].rearrange("p (g c s) -> p g c s", g=NT, s=512)
            KT = [kres.tile([128, S], FP16, name=f"KT{m}") for m in range(NM)]
            QTd = {}       # (m, t) -> [128, 512] fp16
            Xd = {}        # (j, t) -> [128, 512] fp16
            VA = [vaug.tile([128, NHL * 65], FP16, name=f"va{c}")
                  for c in range(NKC)]
            va_view = [va[:].rearrange("p (h c) -> p h c", c=65) for va in VA]
            vwins = {}     # g -> [128, NDC, 128] view of the group window

            def qk_slice(which, t):
                x3, src, eng = ((k3, kT_v, nc.sync) if which == "k"
                                else (q3, qT_v, nc.scalar))
                eng.dma_start(x3[:, t], src[:, t])

            # head-critical loads first, each a single big transfer; the
            # weights ride the gpsimd SWDGE path so they stream in parallel
            # with the k (sync ring) and q (scalar ring) slices
            nc.sync.dma_start(wk_big[:], wkT_d[:])
            nc.scalar.dma_start(wq_big[:], wqT_d[:])
            qk_slice("k", 0)
            qk_slice("q", 0)
            nc.scalar.dma_start(wv_big[:], wvT_d[:])

            # biases, packed [128, n]: one small entry each
            bq_all = const.tile([128, NM], FP32, name="bq_all")
            bk_all = const.tile([128, NM], FP32, name="bk_all")
            bo_all = const.tile([128, NMO], FP32, name="bo_all")
            nc.scalar.dma_start(bq_all[:], bq_d[:])
            nc.scalar.dma_start(bk_all[:], bk_d[:])
            nc.scalar.dma_start(bo_all[:], bo_d[:])
            bq_sb = [bq_all[:, m:m + 1] for m in range(NM)]
            bk_sb = [bk_all[:, m:m + 1] for m in range(NM)]
            bo_sb = [bo_all[:, m:m + 1] for m in range(NMO)]
            # warm the exp activation table before the first real exp
            dummy = const.tile([128, 1], FP16, name="dummy")
            nc.scalar.activation(dummy[:], bq_all[:, 0:1], AF.Exp,
                                 scale=0.125)

            # ---- task emitters -------------------------------------------
            def v_wins(g):
                if g in vwins:
                    return vwins[g]
                w_ = vwinp.tile([128, NDC * 512], FP16, tag="vw", bufs=2,
                                name=f"vw{g}")
                wv3 = w_.rearrange("p (c s) -> p c s", s=512)
                nc.sync.dma_start(wv3, vT_v[:, g])
                vwins[g] = wv3
                return wv3

            done = set()

            def proj(which, m, t):
                """Project m-chunk of token-tile t of q or k."""
                if (which, m, t) in done:
                    return
                done.add((which, m, t))
                xs = k3 if which == "k" else q3
                w_sb = wk_sb if which == "k" else wq_sb
                ps = ps_mm.tile([128, 512], FP32, tag="mm", name="psP")
                for kc in range(NDC):
                    nc.tensor.matmul(
                        ps[:], w_sb[kc][:, ts(m, 128)], xs[:, t, kc, :],
                        start=(kc == 0), stop=(kc == NDC - 1))
                if which == "k":
                    nc.vector.tensor_scalar_add(
                        KT[m][:, ts(t, 512)], ps[:], bk_sb[m])
                else:
                    qt = qtp.tile([128, 512], FP16, tag="qt",
                                  name=f"qt{m}_{t}")
                    QTd[(m, t)] = qt
                    nc.vector.tensor_scalar_add(qt[:], ps[:], bq_sb[m])

            def v_task(c):
                """Project token-chunk c of v into the ones-augmented VA."""
                g, ci = divmod(c, 4)
                vw = v_wins(g)
                ps = ps_mm.tile([128, 512], FP32, tag="mm", name="psV")
                for kc in range(NDC):
                    nc.tensor.matmul(
                        ps[:], vw[:, kc, ts(ci, 128)], wv_sb[kc],
                        start=(kc == 0), stop=(kc == NDC - 1))
                ps_v = ps[:].rearrange("p (h c) -> p h c", c=64)
                nc.vector.tensor_copy(va_view[c][:, :, 0:64], ps_v)
                nc.vector.tensor_copy(va_view[c][:, :, 64:65], onescols[:])

            def out_chunk(t, m):
                """One m-chunk of the output projection for token-tile t."""
                ps = ps_mm.tile([128, 512], FP32, tag="mm", name="psO")
                for j in range(NJ):
                    nc.tensor.matmul(
                        ps[:], wo_sb[j][:, ts(m, 128)], Xd[(j, t)][:],
                        start=(j == 0), stop=(j == NJ - 1))
                st = outstp.tile([128, 512], FP32, tag="st", name="st")
                nc.vector.tensor_scalar_add(st[:], ps[:], bo_sb[m])
                nc.sync.dma_start(out_d[ts(m, 128), ts(t, 512)], st[:])

            # deferred projection fillers, (m, t)-granular; within each m
            # all k-projections first (pair j needs KT[j] complete for all
            # t before its first tile; QT only per-tile)
            filler_q = [(w, m, t) for m in range(1, NM)
                        for w in ("k", "q") for t in range(NT)]

            def pop_filler():
                while filler_q:
                    w, m, t = filler_q.pop(0)
                    if (w, m, t) not in done:
                        proj(w, m, t)
                        return

            # out-projection chunks pending emission (filled after (3,t))
            out_q = []

            def attn_tile(j, t):
                """Attention for head-pair j, token-tile t."""
                first = (j == 0 and t == 0)
                outrow = (j == NJ - 1)
                ys = [ps_y.tile([65, 512], FP32, tag="y", name=f"y{h}")
                      for h in range(2)]
                QTt = QTd[(j, t)]

                def scores(k):
                    s_ps = ps_s.tile([128, 1024], FP32, tag="s", name="s")
                    with tc.high_priority(offset=100000):
                        nc.tensor.matmul(
                            s_ps[:, 0:512], KT[j][0:64, ts(k, 128)],
                            QTt[0:64, :],
                            start=True, stop=True, tile_position=(0, 0))
                        nc.tensor.matmul(
                            s_ps[:, 512:1024], KT[j][64:128, ts(k, 128)],
                            QTt[64:128, :],
                            start=True, stop=True, tile_position=(64, 0))
                    return s_ps

                def a_v(k, p):
                    for h in range(2):
                        nc.tensor.matmul(
                            ys[h][:],
                            VA[k][:, 65 * (2 * j + h):65 * (2 * j + h) + 65],
                            p[:, 512 * h:512 * (h + 1)],
                            start=(k == 0), stop=(k == NKC - 1))

                if first:
                    v_task(0)
                s_cur = scores(0)
                plag = []
                for k in range(NKC):
                    p = ppool.tile([128, 1024], FP16, tag="p", name="p")
                    with tc.high_priority(offset=100000):
                        nc.scalar.activation(p[:], s_cur[:], AF.Exp,
                                             scale=0.125)
                    plag.append((k, p))
                    if k + 1 < NKC:
                        s_cur = scores(k + 1)
                    if first:
                        if k + 1 < NKC:
                            v_task(k + 1)
                        if k in (2, 6, 10):
                            v_wins(k // 4 + 1)
                        if k in (12, 13):
                            # QT(0, t=2..3) ahead of the j=0 row tiles so
                            # their headers don't stall the exp chain
                            proj("q", 0, k - 10)
                    elif outrow and t > 0:
                        # 8 out chunks of the previous token-tile; start at
                        # k=2 so the previous tile's X (j=3) normalization
                        # has landed
                        if k >= 2 and k % 2 == 0 and out_q:
                            out_chunk(*out_q.pop(0))
                            if k == 14 and out_q:
                                out_chunk(*out_q.pop(0))
                        elif k in (3, 7, 11):
                            pop_filler()
                    elif (k in (5, 11) and not outrow) or (
                            outrow and k in (3, 7, 11, 14)):
                        pop_filler()
                    if len(plag) > 2:
                        a_v(*plag.pop(0))
                while plag:
                    a_v(*plag.pop(0))

                # evacuate ys to SBUF (frees the PSUM bank in one DVE op),
                # then normalize entirely from SBUF off the critical path
                xt = xp.tile([128, 512], FP16, tag="x", bufs=16,
                             name=f"x{j}_{t}")
                Xd[(j, t)] = xt
                ybs, rss = [], []
                for h in range(2):
                    rs = small.tile([1, 512], FP32, tag="rs", bufs=1,
                                    name="rs")
                    nc.vector.tensor_copy(rs[:], ys[h][64:65, :])
                    rss.append(rs)
                    yb = ybufp.tile([64, 512], FP32, tag="yb", name="yb")
                    nc.vector.tensor_copy(yb[:], ys[h][0:64, :])
                    ybs.append(yb)
                for h in range(2):
                    rbb = small.tile([64, 512], FP32, tag="rbb", name="rbb")
                    nc.gpsimd.partition_broadcast(
                        rbb[:], rss[h][:], channels=64)
                    ri = small.tile([64, 512], FP32, tag="ri", name="ri")
                    nc.vector.reciprocal_approx_fast(ri[:], rbb[:])
                    nc.vector.tensor_mul(
                        xt[64 * h:64 * h + 64, :], ybs[h][:], ri[:])

            # ---- emission ------------------------------------------------
            v_wins(0)
            for t in range(1, NT):
                qk_slice("k", t)
                qk_slice("q", t)
            for t in range(NT):
                proj("k", 0, t)
            proj("q", 0, 0)
            proj("q", 0, 1)
            for j in range(NJ):
                for t in range(NT):
                    if j == 1 and t == 0:
                        nc.sync.dma_start(wo_big[:], woT_d[:])
                    if t == 0 and j > 0:
                        # KT[j] must be complete (all t-slices) before the
                        # pair's first scores; fillers normally cover this
                        for tt in range(NT):
                            proj("k", j, tt)
                    proj("q", j, t)
                    attn_tile(j, t)
                    if j == NJ - 1:
                        out_q.extend((t, m) for m in range(NMO))
            # drain remaining out chunks (token-tile 3)
            while out_q:
                out_chunk(*out_q.pop(0))

    nc.compile()
    return nc


def _pack_x(xT):
    """[D, S] fp16 -> [128, (t c s)]: per-partition blocks of
    (token-tile, d_in-chunk, 512 tokens), 8KB-contiguous rows."""
    # [D, S] -> [c, 128, t, 512] -> [128, t, c, 512]
    x = xT.reshape(NDC, 128, NT, 512)
    return np.ascontiguousarray(
        x.transpose(1, 2, 0, 3).reshape(128, NT * NDC * 512))


def _pack_w(wT):
    """[Din, N] fp16 -> [128, (c n)]."""
    c = wT.shape[0] // 128
    return np.ascontiguousarray(
        wT.reshape(c, 128, wT.shape[1]).transpose(1, 0, 2)
        .reshape(128, c * wT.shape[1]))


def _prep_in_maps(q, k, v, Wq, bq, Wk, bk, Wv, bv, Wo, bo):
    f16 = np.float16
    in_maps = []
    for core in range(8):
        b, g = divmod(core, G)
        rows = slice(DL * g, DL * (g + 1))
        bo_eff = Wo[:, rows].astype(np.float32) @ bv[rows].astype(np.float32)
        if g == 0:
            bo_eff = bo_eff + bo
        in_maps.append({
            "qT": _pack_x(q[b].T.astype(f16)),
            "kT": _pack_x(k[b].T.astype(f16)),
            "vT": _pack_x(v[b].T.astype(f16)),
            "wqT": _pack_w(Wq[rows, :].T.astype(f16)),
            "wkT": _pack_w(Wk[rows, :].T.astype(f16)),
            "wvT": _pack_w(Wv[rows, :].T.astype(f16)),
            "woT": _pack_w(Wo[:, rows].T.astype(f16)),
            "bq": np.ascontiguousarray(bq[rows].reshape(NM, 128).T
                                       .astype(np.float32)),
            "bk": np.ascontiguousarray(bk[rows].reshape(NM, 128).T
                                       .astype(np.float32)),
            "bo": np.ascontiguousarray(
                bo_eff.astype(np.float32).reshape(NMO, 128).T),
        })
    return in_maps


def kernel(q, k, v, mask, Wq, bq, Wk, bk, Wv, bv, Wo, bo,
           _trace=False, _tmpdir=None):
    from concourse.bass_utils import run_bass_kernel_spmd

    q, k, v = (np.asarray(x, dtype=np.float32) for x in (q, k, v))
    Wq, bq, Wk, bk, Wv, bv, Wo, bo = (
        np.asarray(x, dtype=np.float32)
        for x in (Wq, bq, Wk, bk, Wv, bv, Wo, bo))

    if "nc" not in _CACHED:
        _CACHED["nc"] = _build_nc()
    nc = _CACHED["nc"]

    in_maps = _prep_in_maps(q, k, v, Wq, bq, Wk, bk, Wv, bv, Wo, bo)
    res = run_bass_kernel_spmd(nc, in_maps, list(range(8)), trace=_trace,
                               tmpdir=_tmpdir)
    if _trace:
        _CACHED["last_result"] = res

    out = np.empty((B, S, D), dtype=np.float32)
    for b in range(B):
        acc = res.results[2 * b]["outT"] + res.results[2 * b + 1]["outT"]
        out[b] = acc.T
    return out


# revision 32
# speedup vs baseline: 1.0014x; 1.0014x over previous
"""Trainium2 Bass kernel for nn_MultiHeadAttn (B=4, S=2048, D=1024, H=16).

Sharding: 8 cores = 4 batches x 2 head-groups (tensor-parallel over heads).
Each core computes one batch's attention for 8 of 16 heads (512 of 1024
feature dims) and a partial output projection; the host sums the two
head-group partials per batch (the "all-reduce" of row-parallel Wo).

v2 dataflow (fp16 matmuls, fp32 PSUM):
  - qT/kT are DMA'd ONCE into resident SBUF chunks ([128, 2048] each),
    issued t-slice-major so the first projections overlap the tail of the
    load; vT streams through [128, 512] window tiles.  This removes the 4x
    re-load of q/k windows the m-granular projection fillers used to pay.
  - The exp chain (ScalarE, ~1.06us per [128,1024] tile, 256 tiles) and
    the PE matmul stream (~273us warm) are both ~270us/core: the kernel
    interleaves them so each shadows the other.  Scores for iteration k+1
    and the lag-4 attn@V pair are emitted under exp(k)'s shadow; deferred
    projection m-chunks (1.7us each) fill the remaining PE slack 3 per
    tile; the output projection is emitted as per-m chunks (0.87us), 8
    per tile, inside the j=3 row so it never bunches at tile boundaries.
  - ys ([65,512] fp32 PSUM: 64 attn rows + ones-row sums) is evacuated to
    SBUF by one DVE copy right after the accumulation stops, freeing the
    PSUM bank for the next tile; the softmax division (gpsimd broadcast +
    DVE reciprocal + multiply) then runs entirely from SBUF off the
    critical path.
  - PSUM: 2x proj/V/out accum banks + 2x scores (2 banks each) + 2x ys
    = 8 banks exactly.
"""
import numpy as np

B, S, D = 4, 2048, 1024
H = 16
DK = 64
G = 2              # head groups (tensor-parallel factor)
DL = D // G        # 512 local feature dims per core
NHL = H // G       # 8 local heads
NJ = NHL // 2      # 4 head pairs
NT = S // 512      # 4 token tiles of 512
NKC = S // 128     # 16 k-token chunks of 128
NDC = D // 128     # 8 d_in chunks
NM = DL // 128     # 4 local out chunks
NMO = D // 128     # 8 output d chunks

_CACHED = {}


def _build_nc():
    import concourse.bass as bass
    import concourse.tile as tile
    from concourse import bacc, mybir

    FP32 = mybir.dt.float32
    FP16 = mybir.dt.float16
    AF = mybir.ActivationFunctionType
    ts = bass.ts

    nc = bacc.Bacc(None, target_bir_lowering=False, debug=False)

    # inputs are repacked host-side into the exact SBUF layouts so every
    # DMA is a plain [128, n] copy with 8-32KB contiguous descriptors
    # (1KB descriptors cap a HWDGE ring at ~70-100 GB/s of generation)
    qT_d = nc.dram_tensor("qT", [128, NT * NDC * 512], FP16,
                          kind="ExternalInput")
    kT_d = nc.dram_tensor("kT", [128, NT * NDC * 512], FP16,
                          kind="ExternalInput")
    vT_d = nc.dram_tensor("vT", [128, NT * NDC * 512], FP16,
                          kind="ExternalInput")
    wqT_d = nc.dram_tensor("wqT", [128, NDC * DL], FP16,
                           kind="ExternalInput")
    wkT_d = nc.dram_tensor("wkT", [128, NDC * DL], FP16,
                           kind="ExternalInput")
    wvT_d = nc.dram_tensor("wvT", [128, NDC * DL], FP16,
                           kind="ExternalInput")
    woT_d = nc.dram_tensor("woT", [128, NJ * D], FP16,
                           kind="ExternalInput")
    bq_d = nc.dram_tensor("bq", [128, NM], FP32, kind="ExternalInput")
    bk_d = nc.dram_tensor("bk", [128, NM], FP32, kind="ExternalInput")
    bo_d = nc.dram_tensor("bo", [128, NMO], FP32, kind="ExternalInput")
    out_d = nc.dram_tensor("outT", [D, S], FP32, kind="ExternalOutput")

    with tile.TileContext(nc) as tc:
        with (
            tc.tile_pool(name="const", bufs=1) as const,
            tc.tile_pool(name="wflat", bufs=1) as wflat,
            tc.tile_pool(name="wop", bufs=1) as wop,
            tc.tile_pool(name="xfull", bufs=1) as xfull,
            tc.tile_pool(name="vwinp", bufs=8) as vwinp,
            tc.tile_pool(name="kres", bufs=1) as kres,
            tc.tile_pool(name="qtp", bufs=12) as qtp,
            tc.tile_pool(name="xp", bufs=16) as xp,
            tc.tile_pool(name="vaug", bufs=1) as vaug,
            tc.tile_pool(name="ppool", bufs=5) as ppool,
            tc.tile_pool(name="ybufp", bufs=4) as ybufp,
            tc.tile_pool(name="small", bufs=2) as small,
            tc.tile_pool(name="outstp", bufs=3) as outstp,
            tc.tile_pool(name="ps_mm", bufs=2, space="PSUM") as ps_mm,
            tc.tile_pool(name="ps_s", bufs=2, space="PSUM") as ps_s,
            tc.tile_pool(name="ps_y", bufs=2, space="PSUM") as ps_y,
        ):
            # ---- constants
            onescols = const.tile([128, NHL, 1], FP16, name="onescols")
            nc.vector.memset(onescols[:], 1.0)

            # ---- weight tiles: one flat tile per weight matrix, loaded by
            # a single strided DMA (few big ring entries instead of 8 small
            # ones each — the head phase is gated by DMA ring serialization)
            wk_big = wflat.tile([128, NDC * DL], FP16, name="wk_big")
            wq_big = wflat.tile([128, NDC * DL], FP16, name="wq_big")
            wv_big = wflat.tile([128, NDC * DL], FP16, name="wv_big")
            wo_big = wop.tile([128, NJ * D], FP16, name="wo_big")
            wkv = wk_big[:].rearrange("p (c n) -> p c n", n=DL)
            wqv = wq_big[:].rearrange("p (c n) -> p c n", n=DL)
            wvv = wv_big[:].rearrange("p (c n) -> p c n", n=DL)
            wov = wo_big[:].rearrange("p (c n) -> p c n", n=D)
            wk_sb = [wkv[:, kc, :] for kc in range(NDC)]
            wq_sb = [wqv[:, kc, :] for kc in range(NDC)]
            wv_sb = [wvv[:, kc, :] for kc in range(NDC)]
            wo_sb = [wov[:, jc, :] for jc in range(NJ)]

            # ---- resident activation sources / destinations
            kfull_big = xfull.tile([128, NDC * S], FP16, name="kfull")
            qfull_big = xfull.tile([128, NDC * S], FP16, name="qfull")
            k3 = kfull_big[:].rearrange("p (t c s) -> p t c s", t=NT, s=512)
            q3 = qfull_big[:].rearrange("p (t c s) -> p t c s", t=NT, s=512)
            kT_v = kT_d[:].rearrange("p (t c s) -> p t c s", t=NT, s=512)
            qT_v = qT_d[:].rearrange("p (t c s) -> p t c s", t=NT, s=512)
            vT_v = vT_d[:].rearrange("p (g c s) -> p g c s", g=NT, s=512)
            KT = [kres.tile([128, S], FP16, name=f"KT{m}") for m in range(NM)]
            QTd = {}       # (m, t) -> [128, 512] fp16
            Xd = {}        # (j, t) -> [128, 512] fp16
            VA = [vaug.tile([128, NHL * 65], FP16, name=f"va{c}")
                  for c in range(NKC)]
            va_view = [va[:].rearrange("p (h c) -> p h c", c=65) for va in VA]
            vwins = {}     # g -> [128, NDC, 128] view of the group window

            def qk_slice(which, t):
                x3, src, eng = ((k3, kT_v, nc.sync) if which == "k"
                                else (q3, qT_v, nc.scalar))
                eng.dma_start(x3[:, t], src[:, t])

            # head-critical loads first, each a single big transfer; the
            # weights ride the gpsimd SWDGE path so they stream in parallel
            # with the k (sync ring) and q (scalar ring) slices
            nc.sync.dma_start(wk_big[:], wkT_d[:])
            nc.scalar.dma_start(wq_big[:], wqT_d[:])
            qk_slice("k", 0)
            qk_slice("q", 0)
            nc.scalar.dma_start(wv_big[:], wvT_d[:])

            # biases, packed [128, n]: one small entry each
            bq_all = const.tile([128, NM], FP32, name="bq_all")
            bk_all = const.tile([128, NM], FP32, name="bk_all")
            bo_all = const.tile([128, NMO], FP32, name="bo_all")
            nc.scalar.dma_start(bq_all[:], bq_d[:])
            nc.scalar.dma_start(bk_all[:], bk_d[:])
            nc.scalar.dma_start(bo_all[:], bo_d[:])
            bq_sb = [bq_all[:, m:m + 1] for m in range(NM)]
            bk_sb = [bk_all[:, m:m + 1] for m in range(NM)]
            bo_sb = [bo_all[:, m:m + 1] for m in range(NMO)]
            # warm the exp activation table before the first real exp
            dummy = const.tile([128, 1], FP16, name="dummy")
            nc.scalar.activation(dummy[:], bq_all[:, 0:1], AF.Exp,
                                 scale=0.125)

            # ---- task emitters -------------------------------------------
            def v_wins(g):
                if g in vwins:
                    return vwins[g]
                w_ = vwinp.tile([128, NDC * 512], FP16, tag="vw", bufs=2,
                                name=f"vw{g}")
                wv3 = w_.rearrange("p (c s) -> p c s", s=512)
                nc.sync.dma_start(wv3, vT_v[:, g])
                vwins[g] = wv3
                return wv3

            done = set()

            def proj(which, m, t):
                """Project m-chunk of token-tile t of q or k."""
                if (which, m, t) in done:
                    return
                done.add((which, m, t))
                xs = k3 if which == "k" else q3
                w_sb = wk_sb if which == "k" else wq_sb
                ps = ps_mm.tile([128, 512], FP32, tag="mm", name="psP")
                for kc in range(NDC):
                    nc.tensor.matmul(
                        ps[:], w_sb[kc][:, ts(m, 128)], xs[:, t, kc, :],
                        start=(kc == 0), stop=(kc == NDC - 1))
                if which == "k":
                    nc.vector.tensor_scalar_add(
                        KT[m][:, ts(t, 512)], ps[:], bk_sb[m])
                else:
                    qt = qtp.tile([128, 512], FP16, tag="qt",
                                  name=f"qt{m}_{t}")
                    QTd[(m, t)] = qt
                    nc.vector.tensor_scalar_add(qt[:], ps[:], bq_sb[m])

            def v_task(c):
                """Project token-chunk c of v into the ones-augmented VA."""
                g, ci = divmod(c, 4)
                vw = v_wins(g)
                ps = ps_mm.tile([128, 512], FP32, tag="mm", name="psV")
                for kc in range(NDC):
                    nc.tensor.matmul(
                        ps[:], vw[:, kc, ts(ci, 128)], wv_sb[kc],
                        start=(kc == 0), stop=(kc == NDC - 1))
                ps_v = ps[:].rearrange("p (h c) -> p h c", c=64)
                nc.vector.tensor_copy(va_view[c][:, :, 0:64], ps_v)
                nc.vector.tensor_copy(va_view[c][:, :, 64:65], onescols[:])

            def out_chunk(t, m):
                """One m-chunk of the output projection for token-tile t."""
                ps = ps_mm.tile([128, 512], FP32, tag="mm", name="psO")
                for j in range(NJ):
                    nc.tensor.matmul(
                        ps[:], wo_sb[j][:, ts(m, 128)], Xd[(j, t)][:],
                        start=(j == 0), stop=(j == NJ - 1))
                st = outstp.tile([128, 512], FP32, tag="st", name="st")
                nc.vector.tensor_scalar_add(st[:], ps[:], bo_sb[m])
                nc.sync.dma_start(out_d[ts(m, 128), ts(t, 512)], st[:])

            # deferred projection fillers, (m, t)-granular; within each m
            # all k-projections first (pair j needs KT[j] complete for all
            # t before its first tile; QT only per-tile)
            filler_q = [(w, m, t) for m in range(1, NM)
                        for w in ("k", "q") for t in range(NT)]

            def pop_filler():
                while filler_q:
                    w, m, t = filler_q.pop(0)
                    if (w, m, t) not in done:
                        proj(w, m, t)
                        return

            # out-projection chunks pending emission (filled after (3,t))
            out_q = []

            def attn_tile(j, t):
                """Attention for head-pair j, token-tile t."""
                first = (j == 0 and t == 0)
                outrow = (j == NJ - 1)
                ys = [ps_y.tile([65, 512], FP32, tag="y", name=f"y{h}")
                      for h in range(2)]
                QTt = QTd[(j, t)]

                def scores(k):
                    s_ps = ps_s.tile([128, 1024], FP32, tag="s", name="s")
                    with tc.high_priority(offset=100000):
                        nc.tensor.matmul(
                            s_ps[:, 0:512], KT[j][0:64, ts(k, 128)],
                            QTt[0:64, :],
                            start=True, stop=True, tile_position=(0, 0))
                        nc.tensor.matmul(
                            s_ps[:, 512:1024], KT[j][64:128, ts(k, 128)],
                            QTt[64:128, :],
                            start=True, stop=True, tile_position=(64, 0))
                    return s_ps

                def a_v(k, p):
                    for h in range(2):
                        nc.tensor.matmul(
                            ys[h][:],
                            VA[k][:, 65 * (2 * j + h):65 * (2 * j + h) + 65],
                            p[:, 512 * h:512 * (h + 1)],
                            start=(k == 0), stop=(k == NKC - 1))

                if first:
                    v_task(0)
                s_cur = scores(0)
                plag = []
                for k in range(NKC):
                    p = ppool.tile([128, 1024], FP16, tag="p", name="p")
                    with tc.high_priority(offset=100000):
                        nc.scalar.activation(p[:], s_cur[:], AF.Exp,
                                             scale=0.125)
                    plag.append((k, p))
                    if k + 1 < NKC:
                        s_cur = scores(k + 1)
                    if first:
                        if k + 1 < NKC:
                            v_task(k + 1)
                        if k in (0, 4, 8):
                            # KT[0] slices t=1..3, just ahead of the scores
                            # chunks that read them (and of the DMA slices
                            # still streaming in)
                            proj("k", 0, k // 4 + 1)
                        if k in (2, 6, 10):
                            v_wins(k // 4 + 1)
                        if k in (11, 12, 13):
                            # QT(0, t=1..3) ahead of the j=0 row tiles so
                            # their headers don't stall the exp chain
                            proj("q", 0, k - 10)
                    elif outrow and t > 0:
                        # 8 out chunks of the previous token-tile; start at
                        # k=2 so the previous tile's X (j=3) normalization
                        # has landed
                        if k >= 2 and k % 2 == 0 and out_q:
                            out_chunk(*out_q.pop(0))
                            if k == 14 and out_q:
                                out_chunk(*out_q.pop(0))
                        elif k in (3, 7, 11):
                            pop_filler()
                    elif (k in (5, 11) and not outrow) or (
                            outrow and k in (3, 7, 11, 14)):
                        pop_filler()
                    if len(plag) > 2:
                        a_v(*plag.pop(0))
                while plag:
                    a_v(*plag.pop(0))

                # evacuate ys to SBUF (frees the PSUM bank in one DVE op),
                # then normalize entirely from SBUF off the critical path
                xt = xp.tile([128, 512], FP16, tag="x", bufs=16,
                             name=f"x{j}_{t}")
                Xd[(j, t)] = xt
                ybs, rss = [], []
                for h in range(2):
                    rs = small.tile([1, 512], FP32, tag="rs", bufs=1,
                                    name="rs")
                    nc.vector.tensor_copy(rs[:], ys[h][64:65, :])
                    rss.append(rs)
                    yb = ybufp.tile([64, 512], FP32, tag="yb", name="yb")
                    nc.vector.tensor_copy(yb[:], ys[h][0:64, :])
                    ybs.append(yb)
                for h in range(2):
                    rbb = small.tile([64, 512], FP32, tag="rbb", name="rbb")
                    nc.gpsimd.partition_broadcast(
                        rbb[:], rss[h][:], channels=64)
                    ri = small.tile([64, 512], FP32, tag="ri", name="ri")
                    nc.vector.reciprocal_approx_fast(ri[:], rbb[:])
                    nc.vector.tensor_mul(
                        xt[64 * h:64 * h + 64, :], ybs[h][:], ri[:])

            # ---- emission ------------------------------------------------
            v_wins(0)
            for t in range(1, NT):
                qk_slice("k", t)
                qk_slice("q", t)
            proj("k", 0, 0)
            proj("q", 0, 0)
            for j in range(NJ):
                for t in range(NT):
                    if j == 1 and t == 0:
                        nc.sync.dma_start(wo_big[:], woT_d[:])
                    if t == 0 and j > 0:
                        # KT[j] must be complete (all t-slices) before the
                        # pair's first scores; fillers normally cover this
                        for tt in range(NT):
                            proj("k", j, tt)
                    proj("q", j, t)
                    attn_tile(j, t)
                    if j == NJ - 1:
                        out_q.extend((t, m) for m in range(NMO))
            # drain remaining out chunks (token-tile 3)
            while out_q:
                out_chunk(*out_q.pop(0))

    nc.compile()
    return nc


def _pack_x(xT):
    """[D, S] fp16 -> [128, (t c s)]: per-partition blocks of
    (token-tile, d_in-chunk, 512 tokens), 8KB-contiguous rows."""
    # [D, S] -> [c, 128, t, 512] -> [128, t, c, 512]
    x = xT.reshape(NDC, 128, NT, 512)
    return np.ascontiguousarray(
        x.transpose(1, 2, 0, 3).reshape(128, NT * NDC * 512))


def _pack_w(wT):
    """[Din, N] fp16 -> [128, (c n)]."""
    c = wT.shape[0] // 128
    return np.ascontiguousarray(
        wT.reshape(c, 128, wT.shape[1]).transpose(1, 0, 2)
        .reshape(128, c * wT.shape[1]))


def _prep_in_maps(q, k, v, Wq, bq, Wk, bk, Wv, bv, Wo, bo):
    f16 = np.float16
    in_maps = []
    for core in range(8):
        b, g = divmod(core, G)
        rows = slice(DL * g, DL * (g + 1))
        bo_eff = Wo[:, rows].astype(np.float32) @ bv[rows].astype(np.float32)
        if g == 0:
            bo_eff = bo_eff + bo
        in_maps.append({
            "qT": _pack_x(q[b].T.astype(f16)),
            "kT": _pack_x(k[b].T.astype(f16)),
            "vT": _pack_x(v[b].T.astype(f16)),
            "wqT": _pack_w(Wq[rows, :].T.astype(f16)),
            "wkT": _pack_w(Wk[rows, :].T.astype(f16)),
            "wvT": _pack_w(Wv[rows, :].T.astype(f16)),
            "woT": _pack_w(Wo[:, rows].T.astype(f16)),
            "bq": np.ascontiguousarray(bq[rows].reshape(NM, 128).T
                                       .astype(np.float32)),
            "bk": np.ascontiguousarray(bk[rows].reshape(NM, 128).T
                                       .astype(np.float32)),
            "bo": np.ascontiguousarray(
                bo_eff.astype(np.float32).reshape(NMO, 128).T),
        })
    return in_maps


def kernel(q, k, v, mask, Wq, bq, Wk, bk, Wv, bv, Wo, bo,
           _trace=False, _tmpdir=None):
    from concourse.bass_utils import run_bass_kernel_spmd

    q, k, v = (np.asarray(x, dtype=np.float32) for x in (q, k, v))
    Wq, bq, Wk, bk, Wv, bv, Wo, bo = (
        np.asarray(x, dtype=np.float32)
        for x in (Wq, bq, Wk, bk, Wv, bv, Wo, bo))

    if "nc" not in _CACHED:
        _CACHED["nc"] = _build_nc()
    nc = _CACHED["nc"]

    in_maps = _prep_in_maps(q, k, v, Wq, bq, Wk, bk, Wv, bv, Wo, bo)
    res = run_bass_kernel_spmd(nc, in_maps, list(range(8)), trace=_trace,
                               tmpdir=_tmpdir)
    if _trace:
        _CACHED["last_result"] = res

    out = np.empty((B, S, D), dtype=np.float32)
    for b in range(B):
        acc = res.results[2 * b]["outT"] + res.results[2 * b + 1]["outT"]
        out[b] = acc.T
    return out
